# revision 64
# baseline (speedup 1.0000x reference)
"""Trainium2 Bass kernel for the AttractorNetwork LIF recurrent scan.

Strategy (8 NeuronCores, one chip): pure data-parallel over batch, ZERO
cross-core communication. Each core owns 16 batch rows and keeps the full
[2048, 2048] effective weight matrix in SBUF as bf16 (8 MB). Per timestep:

  rec[b, :] = spk[b, :] @ w_eff        -> 2 column-halves x 16 waves of 4
                                          concurrent matmuls (N=256, M=16)
                                          in PE column groups via
                                          tile_position; each half lands in
                                          its own PSUM bank so the tail
                                          pipelines under the second half
  rec -> rec^T                         -> DVE casts rec to bf16 in 4 chunks;
                                          4 exact 0/1 "selection" matmuls
                                          transpose the 16-row bands back
                                          into the [neuron, batch] state
                                          layout, ACCUMULATING onto membrane
                                          values the DVE pre-wrote into PSUM
                                          (has_written initialized once by
                                          dummy matmuls)
  LIF update on VectorE, 4-way chunked -> spike chunks unblock the next
                                          step's first matmul waves early

The cue is folded into the noise on the host (noise'[t<cue_d] += cue), so
the device loop is: u = mem*decay + noise (+rec via PSUM accumulate);
spk = u >= 1; mem = (spk == 0) * u; acc += spk (second half only). Spikes
are stored as bf16 {0,1} (exact) and feed the next step's matmul as the
stationary operand; weights are bf16 (validated: output identical for the
task's inputs). All eight PSUM banks are allocated full-size so concurrent
PE-writes and DVE-reads never share a bank.

Cores never exchange data, so there is no sensitivity to launch skew or
cross-core latency; the harness gathers per-core [128, 256] activity
accumulators and reassembles the [128, 2048] mean-activity output.
"""

import sys

sys.path.insert(0, "/opt/trn_rl_repo")

import numpy as np
import ml_dtypes

import concourse.bass as bass
import concourse.mybir as mybir
from concourse.bacc import Bacc
from concourse.bass_utils import run_bass_kernel_spmd

F32 = mybir.dt.float32
BF16 = mybir.dt.bfloat16
OP = mybir.AluOpType

N = 2048
B = 128
NCORES = 8
BL = B // NCORES         # 16 batch rows per core
NT = N // 128            # 16 neuron tiles
TAU_MEM = 20.0
DT_ = 1.0
INHIBITION = 0.1
V_THRESH = 1.0
CUE_STRENGTH = 1.0
DECAY = float(np.float32(np.exp(-DT_ / TAU_MEM)))
CHUNK = 4                # noise steps per DMA
RING = 8                 # chunks resident in the SBUF noise ring
F = NT * BL              # 256: state free width ([p, jt*16+b])


def build_nc(T, debug=False, lowering=True, drains=False):
    """Build the (SPMD but communication-free) Bass program for T steps."""
    half = T // 2
    nchunks = (T + CHUNK - 1) // CHUNK

    if lowering:
        nc = Bacc(debug=debug)
    else:
        nc = bass.Bass(debug=debug, target_bir_lowering=False)

    wq = nc.declare_dram_parameter("wq", [128, NT * N], BF16, isOutput=False)
    noise_d = nc.declare_dram_parameter(
        "noise", [nchunks, 128, CHUNK * F], F32, isOutput=False)
    ident_d = nc.declare_dram_parameter("ident_d", [128, 64], BF16, isOutput=False)
    out_d = nc.declare_dram_parameter("out", [128, F], F32, isOutput=True)

    from contextlib import ExitStack
    with ExitStack() as es:
        w_sb = es.enter_context(nc.sbuf_tensor("w_sb", [128, NT * N], BF16))
        ring = es.enter_context(
            nc.sbuf_tensor("ring", [128, RING * CHUNK * F], F32))
        rec_sb = es.enter_context(nc.sbuf_tensor("rec_sb", [128, 512], BF16))
        ident = es.enter_context(nc.sbuf_tensor("ident", [128, 64], BF16))
        zeros = es.enter_context(nc.sbuf_tensor("zeros", [128, 128], BF16))
        spk0 = es.enter_context(nc.sbuf_tensor("spk0", [128, F], BF16))
        spk1 = es.enter_context(nc.sbuf_tensor("spk1", [128, F], BF16))
        mem = es.enter_context(nc.sbuf_tensor("mem", [128, F], F32))
        u = es.enter_context(nc.sbuf_tensor("u", [128, F], F32))
        acc = es.enter_context(nc.sbuf_tensor("acc", [128, F], F32))
        # 8 full PSUM banks: rec[parity][half] + spk[parity][pair]; full-
        # bank allocation keeps concurrent PE-writes and DVE-reads in
        # different banks (same-bank PE-W + DVE-R is a hard fault)
        psrec_t = [[es.enter_context(
            nc.psum_tensor(f"psr{p}{h}", [128, 512], F32))
            for h in range(2)] for p in range(2)]
        psspk_t = [[es.enter_context(
            nc.psum_tensor(f"pss{p}{h}", [128, 512], F32))
            for h in range(2)] for p in range(2)]
        w_sem = es.enter_context(nc.semaphore("w_sem"))
        noise_rdy = [
            es.enter_context(nc.semaphore(f"noise_rdy{i}")) for i in range(RING)
        ]
        noise_cons = es.enter_context(nc.semaphore("noise_cons"))
        mm_done = es.enter_context(nc.semaphore("mm_done"))
        copy_sem = es.enter_context(nc.semaphore("copy_sem"))
        tp_done = es.enter_context(nc.semaphore("tp_done"))
        spk_own = es.enter_context(nc.semaphore("spk_own"))
        u_done = es.enter_context(nc.semaphore("u_done"))
        init_done = es.enter_context(nc.semaphore("init_done"))
        odma = es.enter_context(nc.semaphore("odma"))
        block = es.enter_context(nc.Block())

        spk_buf = [spk0, spk1]

        def noise_ap(t):
            c = (t % (RING * CHUNK)) * F
            return ring[:, c:c + F]

        @block.sync
        def _(sync):
            sync.dma_start(out=ident[:, :], in_=ident_d[:, :]).then_inc(w_sem, 16)
            sync.dma_start(out=w_sb[:, :], in_=wq[:, :]).then_inc(w_sem, 16)
            for c in range(nchunks):
                if c >= RING:
                    sync.wait_ge(noise_cons, (c - RING) * CHUNK + CHUNK)
                s = (c % RING) * CHUNK * F
                sync.dma_start(
                    out=ring[:, s:s + CHUNK * F], in_=noise_d[c]
                ).then_inc(noise_rdy[c % RING], 16)
            sync.wait_ge(noise_cons, T)
            sync.dma_start(out=out_d[:, :], in_=acc[:, :]).then_inc(odma, 16)
            sync.wait_ge(odma, 16)

        @block.tensor
        def _(tensor):
            tensor.wait_ge(w_sem, 32)
            tensor.wait_ge(init_done, 1)
            # dummy start=True matmuls: set the has_written bits of every
            # used ps_spk element once, so later sel-matmuls with
            # start=False ACCUMULATE onto DVE-written membrane values (the
            # documented cayman DVE-write + matmul-accumulate workaround).
            # One single-group matmul per bank (start clears the whole
            # bank's bits).
            for p in range(2):
                for pr in range(2):
                    dm = tensor.matmul(
                        psspk_t[p][pr][:, 0:128],
                        zeros[:, :],
                        spk_buf[p][:, 0:128],
                        start=True, stop=True,
                        skip_group_check=True,
                    )
            dm.then_inc(init_done, 1)
            for t in range(1, T):
                par = t % 2
                ppar = (t - 1) % 2

                def sel(cc, tt):
                    # rec chunk cc of step tt -> state layout, accumulated
                    # onto the pre-written membrane
                    ptt = tt % 2
                    tensor.wait_ge(copy_sem, 4 * (tt - 1) + cc + 1)
                    if cc % 2 == 0:
                        tensor.wait_ge(u_done, 2 * (tt - 1) + cc // 2 + 1)
                    tp = tensor.matmul(
                        psspk_t[ptt][cc // 2][:, 64 * (cc % 2):
                                              64 * (cc % 2) + 64],
                        rec_sb[:, 128 * cc:128 * cc + 128],
                        ident[:, :],
                        start=False, stop=True,
                        skip_group_check=True,
                    )
                    if cc % 2 == 1:
                        tp.then_inc(tp_done, 1)

                # asymmetric halves (384/128 cols): casts for chunks 0-2
                # finish during the short second half, leaving only cast3
                # exposed (hidden behind sel0-2)
                HW_ = (384, 128)

                def wave(h2, i, pv, buf):
                    lcol = 64 * (i % 4) + BL * (i // 4)
                    off = 0 if h2 == 0 else HW_[0]
                    wcol = N * i + off
                    for g in range(4):
                        mm = tensor.matmul(
                            psrec_t[pv][h2][32 * g:32 * g + BL, 0:HW_[h2]],
                            buf[:, lcol:lcol + BL],
                            w_sb[:, wcol + 512 * g:
                                 wcol + 512 * g + HW_[h2]],
                            start=(i == 0),
                            stop=(i == NT - 1),
                            tile_position=(0, 32 * g),
                            skip_group_check=True,
                        )
                    return mm

                # i-tile order groups waves by spike chunk (i%4) so each
                # is_ge chunk feeds four consecutive waves: the DVE gets
                # ~4 wave-times of slack per chunk instead of one
                order = [i4 + 4 * q for i4 in range(4) for q in range(4)]
                for idx, i in enumerate(order):
                    if idx % 4 == 0:
                        tensor.wait_ge(spk_own, 4 * (t - 1) + idx // 4 + 1)
                    mm = wave(0, i, par, spk_buf[ppar])
                mm.then_inc(mm_done, 1)
                # right half
                for i in order:
                    mm = wave(1, i, par, spk_buf[ppar])
                mm.then_inc(mm_done, 1)
                for cc in range(4):
                    sel(cc, t)

        @block.vector
        def _(vector):
            vector.memset(acc[:, :], 0.0)
            # zero the never-written partition bands of the matmul PSUM so
            # the rec copy reads defined data
            for p in range(2):
                for h in range(2):
                    vector.memset(psrec_t[p][h][:, :], 0.0)
            vector.memset(spk_buf[0][:, :], 0.0)
            vector.memset(spk_buf[1][:, :], 0.0)
            vector.memset(zeros[:, :], 0.0).then_inc(init_done, 1)
            vector.wait_ge(init_done, 2)
            # t = 0: mem was 0, rec = 0 -> membrane is just noise'(0)
            vector.wait_ge(noise_rdy[0], 16)
            na = noise_ap(0)
            vector.tensor_scalar(
                spk_buf[0][:, :], na, V_THRESH, None, OP.is_ge
            ).then_inc(spk_own, 4)
            vector.scalar_tensor_tensor(
                mem[:, :], na, V_THRESH, na, OP.is_lt, OP.mult
            ).then_inc(noise_cons, 1)

            for t in range(1, T):
                par = t % 2
                if t % CHUNK == 0:
                    c = t // CHUNK
                    vector.wait_ge(noise_rdy[c % RING], 16 * (c // RING + 1))
                if drains:
                    vector.drain()   # order: mem/acc writes of step t-1
                # pre-write the leak+noise membrane into the spike PSUM
                # pairs; the sel-matmuls accumulate rec^T on top
                for pr in range(2):
                    ns = noise_ap(t)[:, 128 * pr:128 * pr + 128]
                    vector.scalar_tensor_tensor(
                        psspk_t[par][pr][:, 0:128],
                        mem[:, 128 * pr:128 * pr + 128], DECAY, ns,
                        OP.mult, OP.add).then_inc(u_done, 1)
                for cc in range(4):
                    if cc == 0:
                        # chunks 0-2 need the 384-col half, chunk 3 the rest
                        vector.wait_ge(mm_done, 2 * (t - 1) + 1)
                    elif cc == 3:
                        vector.wait_ge(mm_done, 2 * (t - 1) + 2)
                    # rec copy+cast chunk cc feeds sel-matmul chunk cc
                    vector.tensor_copy(
                        rec_sb[:, 128 * cc:128 * cc + 128],
                        psrec_t[par][0 if cc < 3 else 1][
                            :, 128 * cc if cc < 3 else 0:
                            (128 * cc if cc < 3 else 0) + 128],
                    ).then_inc(copy_sem, 1)
                for cc in range(4):
                    # spike chunk cc unblocks next step's matmul wave cc
                    if cc % 2 == 0:
                        vector.wait_ge(tp_done, 2 * (t - 1) + cc // 2 + 1)
                    vector.tensor_scalar(
                        spk_buf[par][:, 64 * cc:64 * cc + 64],
                        psspk_t[par][cc // 2][:, 64 * (cc % 2):
                                              64 * (cc % 2) + 64],
                        V_THRESH, None, OP.is_ge
                    ).then_inc(spk_own, 1)
                if drains:
                    vector.drain()   # order: spk write before mem gate read
                for pr in range(2):
                    if t >= half:
                        vector.scalar_tensor_tensor(
                            acc[:, 128 * pr:128 * pr + 128],
                            psspk_t[par][pr][:, 0:128], V_THRESH,
                            acc[:, 128 * pr:128 * pr + 128],
                            OP.is_ge, OP.add)
                    # mem = u * (u < 1) == (spk == 0) * u, one PSUM read
                    st = vector.scalar_tensor_tensor(
                        mem[:, 128 * pr:128 * pr + 128],
                        spk_buf[par][:, 128 * pr:128 * pr + 128], 0.0,
                        psspk_t[par][pr][:, 0:128], OP.is_equal, OP.mult)
                st.then_inc(noise_cons, 1)

    return nc


def prep_inputs(cue, weights, noise, T, cue_duration):
    """Host-side sharding: returns in_maps for run_bass_kernel_spmd."""
    cue = np.asarray(cue, np.float32)
    weights = np.asarray(weights, np.float32)
    noise = np.asarray(noise, np.float32)

    w_eff = (weights - np.float32(INHIBITION / N)) * (
        1.0 - np.eye(N, dtype=np.float32))

    noise_eff = noise.copy()
    noise_eff[:cue_duration] += np.float32(CUE_STRENGTH) * cue

    nchunks = (T + CHUNK - 1) // CHUNK
    pad = nchunks * CHUNK - T
    if pad:
        noise_eff = np.concatenate(
            [noise_eff, np.zeros((pad, B, N), np.float32)], axis=0)

    # replicated weights: wq[p, i_tile*N + j] = w_eff[128*i_tile + p, j]
    wq = np.ascontiguousarray(
        w_eff.reshape(NT, 128, N).transpose(1, 0, 2).reshape(128, NT * N)
    ).astype(ml_dtypes.bfloat16)

    # 0/1 selection matrix for the rec-band transpose matmuls:
    # column 16g+b picks row 32g+b
    ident = np.zeros((128, 64), ml_dtypes.bfloat16)
    for g in range(4):
        for b in range(BL):
            ident[32 * g + b, 16 * g + b] = 1.0

    in_maps = []
    for r in range(NCORES):
        bsl = slice(BL * r, BL * r + BL)
        # noise: [t, b, j] -> [chunk, p, (q, cc, g, b)] where the state free
        # order is (cc, g, b) with neuron tile jt = 4g + cc
        nz = noise_eff[:, bsl, :]                     # [Tp, 16, 2048]
        nz = nz.transpose(0, 2, 1)                    # [Tp, 2048j, 16b]
        nz = nz.reshape(-1, 4, 4, 128, BL)            # [Tp, g, cc, p, b]
        nz = nz.transpose(0, 3, 2, 1, 4)              # [Tp, p, cc, g, b]
        nz = nz.reshape(nchunks, CHUNK, 128, F)       # [c, q, p, f]
        nz = nz.transpose(0, 2, 1, 3)                 # [c, p, q, f]
        nz = np.ascontiguousarray(
            nz.reshape(nchunks, 128, CHUNK * F), dtype=np.float32)
        in_maps.append({"wq": wq, "noise": nz, "ident_d": ident})
    return in_maps


def assemble_output(outs, T):
    """outs: per-core {"out": [128, 256]} -> [B, N] mean activity."""
    half = T // 2
    mean = np.empty((B, N), np.float32)
    for r in range(NCORES):
        oc = np.asarray(outs[r]["out"], np.float32)   # [p, 64cc+16g+b]
        oc = oc.reshape(128, 4, 4, BL)                # [p, cc, g, b]
        blk = oc.transpose(3, 2, 1, 0).reshape(BL, N)  # [b, (g, cc, p)]
        mean[BL * r:BL * r + BL, :] = blk
    return mean / np.float32(half)


_NC_CACHE = {}


def _ensure_ntff_hook():
    """The agent image's antenv lacks axon_hooks; recreate it so
    run_bass_kernel_spmd(trace=True) can capture NTFF profiles."""
    import types
    import ctypes
    import contextlib
    try:
        from antenv.axon_hooks import get_axon_ntff_profile_hook  # noqa: F401
        return
    except ImportError:
        pass
    so_path = "/opt/axon/libaxon_pjrt.so"
    try:
        lib = ctypes.CDLL(so_path)
        if not hasattr(lib, "axon_start_nrt_profile"):
            return
    except OSError:
        return
    lib.axon_start_nrt_profile.argtypes = [
        ctypes.POINTER(ctypes.c_int64), ctypes.c_size_t]
    lib.axon_start_nrt_profile.restype = ctypes.c_int64
    lib.axon_stop_nrt_profile.argtypes = [ctypes.c_char_p]
    lib.axon_stop_nrt_profile.restype = ctypes.c_int64

    @contextlib.contextmanager
    def _hook(output_dir, device_ids):
        import jax
        jax.devices()
        if device_ids:
            ids = (ctypes.c_int64 * len(device_ids))(*device_ids)
            rc = lib.axon_start_nrt_profile(ids, len(device_ids))
        else:
            rc = lib.axon_start_nrt_profile(None, 0)
        if rc != 0:
            raise RuntimeError(f"axon_start_nrt_profile rc={rc}")
        try:
            yield
        finally:
            n = lib.axon_stop_nrt_profile(str(output_dir).encode())
            if n < 0:
                raise RuntimeError(f"axon_stop_nrt_profile rc={n}")

    mod = types.ModuleType("antenv.axon_hooks")
    mod._hook = _hook
    mod.get_axon_ntff_profile_hook = lambda: mod._hook
    mod.set_axon_ntff_profile_hook = lambda h: setattr(mod, "_hook", h)
    sys.modules["antenv.axon_hooks"] = mod


def kernel(cue, weights, noise, steps, cue_duration, trace=False):
    T = int(steps)
    cd = int(cue_duration)
    in_maps = prep_inputs(cue, weights, noise, T, cd)
    if T not in _NC_CACHE:
        nc_new = build_nc(T)
        nc_new.finalize()
        _NC_CACHE[T] = nc_new
    nc = _NC_CACHE[T]
    if trace:
        _ensure_ntff_hook()
    res = run_bass_kernel_spmd(nc, in_maps, list(range(NCORES)), trace=trace)
    out = assemble_output(res.results, T)
    kernel.last_result = res
    return out
